# revision 7
# baseline (speedup 1.0000x reference)
"""Trainium2 Bass kernel for nn_Mix9Net (directional-conv resnet).

Strategy: data-parallel over batch across 8 NeuronCores (32 images/core).
Each core runs all 4 board directions sequentially (shared weights,
different 3-tap shift offsets).

Layout: activations live in SBUF as [128 ch, 32 img * 17*17] float32r —
each 15x15 image padded with a zero ring so the 3-tap directional convs
become three accumulating matmuls whose rhs APs are the interior view
shifted by a constant flat offset (dr*17+dc).  fp32r runs at full PE rate
for N>=256 but requires an even innermost AP count, so conv rhs reads 16
columns per row (15 interior + 1 ring column whose results are dropped
when ScalarE reads the PSUM banks back with a [4,30,15]-of-16 AP).
Silu+bias is fused into ScalarE activations covering 4 PSUM banks per op;
residual adds run on VectorE in-place over 8-image interior groups so the
layer pipeline keeps flowing.
"""
import numpy as np

import concourse.bass as bass
import concourse.tile as tile
from concourse import bacc, mybir
from concourse.bass_utils import run_bass_kernel_spmd

f32 = mybir.dt.float32
f32r = mybir.dt.float32r

B, C_IN, H, W = 256, 2, 15, 15
DIM_MID, DIM_OUT = 128, 64
N_RES = 4
N_CORES = 8
NB = B // N_CORES            # 32 images per core
PH, PW = H + 2, W + 2        # 17x17 padded image
PAD = PH * PW                # 289
SPAN = NB * PAD              # 9248
G = 18                       # head guard (shifted AP offsets stay >= 0)
GT = 20                      # tail guard (junk col reads past last image)
BUF = G + SPAN + GT
NI = NB * H * W              # 7200 interior elems per partition
CHUNK = 2 * H * W            # 450 = 2 images per flat matmul chunk
WCHUNK = 2 * H * 16          # 480 = 2 images of 16-wide conv rhs
NCHUNK = NI // CHUNK         # 16 chunks
GRP = 4                      # chunks per PSUM-tile group (4 banks)
NGRP = NCHUNK // GRP         # 4 groups of 8 images

# tap positions in the 3x3 kernel per direction (matches reference _POS):
# horiz, vert, diag, anti-diag.  Tap k reads input offset (r-1, c-1).
POS = (((1, 0), (1, 1), (1, 2)),
       ((0, 1), (1, 1), (2, 1)),
       ((0, 0), (1, 1), (2, 2)),
       ((2, 0), (1, 1), (0, 2)))
OFFS = [tuple((r - 1) * PW + (c - 1) for r, c in taps) for taps in POS]

SILU = mybir.ActivationFunctionType.Silu


def _imgs(t, off=0):
    """[128, NB, 17, 17] padded-image view of stream buffer, shifted."""
    s = G + off
    return t[:, s:s + SPAN].rearrange("p (i h w) -> p i h w", i=NB, h=PH, w=PW)


def _conv_rhs(t, off, c):
    """[128, 2, 15, 16] moving operand for chunk c (images 2c, 2c+1):
    15 interior rows, 16 cols (last col junk, even count for fp32r)."""
    return _imgs(t, off)[:, 2 * c:2 * c + 2, 1:16, 1:17]


def _valid(ps, parts=DIM_MID):
    """[p, 4, 30, 15] valid-column view of 4 PSUM banks of 480."""
    return ps[0:parts, :, 0:WCHUNK].rearrange(
        "p b (rr c) -> p b rr c", rr=2 * H, c=16)[:, :, :, 0:15]


def _int8(t, g):
    """[128, 8, 15, 15] interior of image group g."""
    return _imgs(t)[:, 8 * g:8 * g + 8, 1:16, 1:16]


def build_program(act_func=SILU, repeats=1):
    nc = bacc.Bacc("TRN2", target_bir_lowering=False, debug=False)

    x3_d = nc.dram_tensor("x3", [4, 6, NI], f32r, kind="ExternalInput").ap()
    w03_d = nc.dram_tensor("w03", [6, DIM_MID], f32r, kind="ExternalInput").ap()
    b0_d = nc.dram_tensor("b0", [DIM_MID, 1], f32, kind="ExternalInput").ap()
    rbw_d = nc.dram_tensor("rbw", [N_RES, 3, DIM_MID, DIM_MID], f32r, kind="ExternalInput").ap()
    rbb_d = nc.dram_tensor("rbb", [N_RES, DIM_MID, 1], f32, kind="ExternalInput").ap()
    rbc1_d = nc.dram_tensor("rbc1", [N_RES, DIM_MID, DIM_MID], f32r, kind="ExternalInput").ap()
    rbc1b_d = nc.dram_tensor("rbc1b", [N_RES, DIM_MID, 1], f32, kind="ExternalInput").ap()
    c0w1_d = nc.dram_tensor("c0w1", [DIM_MID, DIM_MID], f32r, kind="ExternalInput").ap()
    c0b1_d = nc.dram_tensor("c0b1", [DIM_MID, 1], f32, kind="ExternalInput").ap()
    c0w2_d = nc.dram_tensor("c0w2", [DIM_MID, DIM_MID], f32r, kind="ExternalInput").ap()
    c0b2_d = nc.dram_tensor("c0b2", [DIM_MID, 1], f32, kind="ExternalInput").ap()
    finw_d = nc.dram_tensor("finw", [DIM_MID, DIM_OUT], f32r, kind="ExternalInput").ap()
    finb_d = nc.dram_tensor("finb", [DIM_OUT, 1], f32, kind="ExternalInput").ap()
    zb_d = nc.dram_tensor("zbuf", [DIM_MID, PAD], f32r, kind="ExternalInput").ap()
    out_d = nc.dram_tensor("out", [4, DIM_OUT, NI], f32, kind="ExternalOutput").ap()

    with tile.TileContext(nc) as tc:
        with (
            tc.tile_pool(name="const", bufs=1) as cpool,
            tc.tile_pool(name="stream", bufs=1) as spool,
            tc.tile_pool(name="x3p", bufs=1) as x3pool,
            tc.tile_pool(name="psum", bufs=2, space="PSUM") as psum,
        ):
            # ---- constants (loaded once) ----
            w03 = cpool.tile([6, DIM_MID], f32r)
            rbw = cpool.tile([DIM_MID, N_RES * 3, DIM_MID], f32r)
            rbc1 = cpool.tile([DIM_MID, N_RES, DIM_MID], f32r)
            c0w1 = cpool.tile([DIM_MID, DIM_MID], f32r)
            c0w2 = cpool.tile([DIM_MID, DIM_MID], f32r)
            finw = cpool.tile([DIM_MID, DIM_OUT], f32r)
            b0 = cpool.tile([DIM_MID, 1], f32)
            rbb = cpool.tile([DIM_MID, N_RES], f32)
            rbc1b = cpool.tile([DIM_MID, N_RES], f32)
            c0b1 = cpool.tile([DIM_MID, 1], f32)
            c0b2 = cpool.tile([DIM_MID, 1], f32)
            finb = cpool.tile([DIM_OUT, 1], f32)

            nc.sync.dma_start(w03[:], w03_d)
            nc.sync.dma_start(rbw[:], rbw_d.rearrange("i k ci co -> ci (i k) co"))
            nc.sync.dma_start(rbc1[:], rbc1_d.rearrange("i ci co -> ci i co"))
            nc.sync.dma_start(c0w1[:], c0w1_d)
            nc.sync.dma_start(c0w2[:], c0w2_d)
            nc.sync.dma_start(finw[:], finw_d)
            nc.sync.dma_start(b0[:], b0_d)
            nc.sync.dma_start(rbb[:], rbb_d.rearrange("i p one -> p (i one)"))
            nc.sync.dma_start(rbc1b[:], rbc1b_d.rearrange("i p one -> p (i one)"))
            nc.sync.dma_start(c0b1[:], c0b1_d)
            nc.sync.dma_start(c0b2[:], c0b2_d)
            nc.sync.dma_start(finb[:], finb_d)

            # ---- streams ----
            s_t = spool.tile([DIM_MID, BUF], f32r)  # residual stream (padded)
            u_t = spool.tile([DIM_MID, NI], f32r)   # block branch (flat)
            t_t = spool.tile([DIM_MID, NI], f32r)   # dconv-out (flat)
            o_t = spool.tile([DIM_OUT, NI], f32)    # final out (flat)

            # zero the stream buffer (ring + guards) via repeated zbuf DMAs
            nc.sync.dma_start(s_t[:, 0:G], zb_d[:, 0:G])
            for i in range(NB):
                nc.sync.dma_start(s_t[:, G + i * PAD:G + (i + 1) * PAD], zb_d)
            nc.sync.dma_start(s_t[:, G + SPAN:], zb_d[:, 0:GT])

            def act_group(dst, src, bias_ap):
                nc.scalar.activation(dst, src, act_func, bias=bias_ap)

            for d in range(4 * repeats):
                d = d % 4
                offs = OFFS[d]
                x3_t = x3pool.tile([6, NI], f32r)
                nc.sync.dma_start(x3_t[:], x3_d[d])

                # dconv0: K=6 matmul over pre-shifted input copies
                for g in range(NGRP):
                    ps = psum.tile([DIM_MID, GRP, 512], f32, tag="ps")
                    for j in range(GRP):
                        c = GRP * g + j
                        nc.tensor.matmul(
                            ps[:, j, 0:CHUNK], w03[:],
                            x3_t[:, c * CHUNK:(c + 1) * CHUNK])
                    act_group(_int8(s_t, g), ps[:, :, 0:CHUNK], b0[:])

                # 4 directional-conv res blocks
                for i in range(N_RES):
                    for g in range(NGRP):
                        ps = psum.tile([DIM_MID, GRP, 512], f32, tag="ps")
                        for j in range(GRP):
                            c = GRP * g + j
                            for k in range(3):
                                nc.tensor.matmul(
                                    ps[:, j, 0:WCHUNK], rbw[:, 3 * i + k, :],
                                    _conv_rhs(s_t, offs[k], c),
                                    start=(k == 0), stop=(k == 2))
                        act_group(t_t[:, g * GRP * CHUNK:(g + 1) * GRP * CHUNK],
                                  _valid(ps), rbb[:, i:i + 1])
                    for g in range(NGRP):
                        ps = psum.tile([DIM_MID, GRP, 512], f32, tag="ps")
                        for j in range(GRP):
                            c = GRP * g + j
                            nc.tensor.matmul(
                                ps[:, j, 0:CHUNK], rbc1[:, i, :],
                                t_t[:, c * CHUNK:(c + 1) * CHUNK])
                        act_group(u_t[:, g * GRP * CHUNK:(g + 1) * GRP * CHUNK],
                                  ps[:, :, 0:CHUNK], rbc1b[:, i:i + 1])
                        nc.vector.tensor_add(
                            _int8(s_t, g), _int8(s_t, g),
                            u_t[:, g * GRP * CHUNK:(g + 1) * GRP * CHUNK].rearrange(
                                "p (i h w) -> p i h w", i=8, h=H, w=W))

                # Conv0dResBlock: two pointwise layers + residual
                for g in range(NGRP):
                    ps = psum.tile([DIM_MID, GRP, 512], f32, tag="ps")
                    for j in range(GRP):
                        c = GRP * g + j
                        nc.tensor.matmul(
                            ps[:, j, 0:WCHUNK], c0w1[:],
                            _conv_rhs(s_t, 0, c))
                    act_group(t_t[:, g * GRP * CHUNK:(g + 1) * GRP * CHUNK],
                              _valid(ps), c0b1[:])
                for g in range(NGRP):
                    ps = psum.tile([DIM_MID, GRP, 512], f32, tag="ps")
                    for j in range(GRP):
                        c = GRP * g + j
                        nc.tensor.matmul(
                            ps[:, j, 0:CHUNK], c0w2[:],
                            t_t[:, c * CHUNK:(c + 1) * CHUNK])
                    act_group(u_t[:, g * GRP * CHUNK:(g + 1) * GRP * CHUNK],
                              ps[:, :, 0:CHUNK], c0b2[:])
                    nc.vector.tensor_add(
                        _int8(s_t, g), _int8(s_t, g),
                        u_t[:, g * GRP * CHUNK:(g + 1) * GRP * CHUNK].rearrange(
                            "p (i h w) -> p i h w", i=8, h=H, w=W))

                # final 1x1 conv (64 out channels) + bias
                for g in range(NGRP):
                    ps = psum.tile([DIM_MID, GRP, 512], f32, tag="ps")
                    for j in range(GRP):
                        c = GRP * g + j
                        nc.tensor.matmul(
                            ps[0:DIM_OUT, j, 0:WCHUNK], finw[:],
                            _conv_rhs(s_t, 0, c))
                    nc.vector.tensor_scalar_add(
                        o_t[:, g * GRP * CHUNK:(g + 1) * GRP * CHUNK],
                        _valid(ps, DIM_OUT), finb[:])
                nc.sync.dma_start(out_d[d], o_t[:])

    nc.compile()
    return nc


def prep_shared_inputs(dconv0_w, dconv0_b, rb_dconv_w, rb_dconv_b, rb_c1_w,
                       rb_c1_b, c0_w1, c0_b1, c0_w2, c0_b2, final_w, final_b):
    """Host-side weight transposes shared by all cores."""
    f = np.float32
    w03 = np.ascontiguousarray(
        np.asarray(dconv0_w, f).transpose(0, 2, 1).reshape(6, DIM_MID))
    return {
        "w03": _round_f32r(w03),
        "b0": np.asarray(dconv0_b, f).reshape(DIM_MID, 1),
        "rbw": _round_f32r(np.ascontiguousarray(
            np.asarray(rb_dconv_w, f).transpose(0, 1, 3, 2))),
        "rbb": np.asarray(rb_dconv_b, f).reshape(N_RES, DIM_MID, 1),
        "rbc1": _round_f32r(np.ascontiguousarray(
            np.asarray(rb_c1_w, f).transpose(0, 2, 1))),
        "rbc1b": np.asarray(rb_c1_b, f).reshape(N_RES, DIM_MID, 1),
        "c0w1": _round_f32r(np.ascontiguousarray(np.asarray(c0_w1, f).T)),
        "c0b1": np.asarray(c0_b1, f).reshape(DIM_MID, 1),
        "c0w2": _round_f32r(np.ascontiguousarray(np.asarray(c0_w2, f).T)),
        "c0b2": np.asarray(c0_b2, f).reshape(DIM_MID, 1),
        "finw": _round_f32r(np.ascontiguousarray(np.asarray(final_w, f).T)),
        "finb": np.asarray(final_b, f).reshape(DIM_OUT, 1),
        "zbuf": np.zeros((DIM_MID, PAD), f),
    }


def _round_f32r(a):
    """RNE-round fp32 to fp32r (1s+8e+11m, top 20 bits) on host, so HW
    truncation of the low 12 bits loses nothing."""
    v = np.ascontiguousarray(a, np.float32).view(np.uint32)
    r = (v + 0x7FF + ((v >> 12) & 1)) & np.uint32(0xFFFFF000)
    return r.astype(np.uint32).view(np.float32)


def prep_x3(x_shard):
    """[NB, 2, 15, 15] -> [4, 6, NI] pre-shifted interior copies."""
    P = np.zeros((NB, C_IN, PH, PW), np.float32)
    P[:, :, 1:16, 1:16] = x_shard
    x3 = np.empty((4, 6, NI), np.float32)
    for d, taps in enumerate(POS):
        for k, (sr, sc) in enumerate(taps):
            sh = P[:, :, sr:sr + H, sc:sc + W]          # [NB, 2, 15, 15]
            x3[d, 2 * k:2 * k + 2] = sh.transpose(1, 0, 2, 3).reshape(C_IN, NI)
    return _round_f32r(x3)


_CACHE = {}


def kernel(**inputs):
    if "nc" not in _CACHE:
        _CACHE["nc"] = build_program()
    nc = _CACHE["nc"]

    x = np.asarray(inputs["x"], np.float32)
    shared = prep_shared_inputs(**{k: v for k, v in inputs.items() if k != "x"})

    in_maps = []
    for c in range(N_CORES):
        shard = x[c * NB:(c + 1) * NB]
        m = dict(shared)
        m["x3"] = prep_x3(shard)
        in_maps.append(m)

    res = run_bass_kernel_spmd(nc, in_maps, core_ids=list(range(N_CORES)))

    out = np.empty((B, 4, DIM_OUT, H, W), np.float32)
    for c in range(N_CORES):
        oc = res.results[c]["out"].reshape(4, DIM_OUT, NB, H, W)
        out[c * NB:(c + 1) * NB] = oc.transpose(2, 0, 1, 3, 4)
    return out


# revision 10
# speedup vs baseline: 208.1350x; 208.1350x over previous
"""Trainium2 Bass kernel for nn_Mix9Net (directional-conv resnet).

Strategy: data-parallel over batch across 8 NeuronCores (32 images/core).
Each core runs all 4 board directions sequentially (shared weights,
different 3-tap shift offsets).

Layout: activations live in SBUF as [128 ch, 32 img * 17*17] float32r —
each 15x15 image padded with a zero ring so the 3-tap directional convs
become three accumulating matmuls whose rhs APs are the interior view
shifted by a constant flat offset (dr*17+dc).  fp32r runs at full PE rate
for N>=256 but requires an even innermost AP count, so conv rhs reads 16
columns per row (15 interior + 1 ring column whose results are dropped
when ScalarE reads the PSUM banks back with a [4,30,15]-of-16 AP).
Silu+bias is fused into ScalarE activations covering 4 PSUM banks per op;
residual adds run on VectorE in-place over 8-image interior groups so the
layer pipeline keeps flowing.
"""
import numpy as np

import concourse.bass as bass
import concourse.tile as tile
from concourse import bacc, mybir
from concourse.bass_utils import run_bass_kernel_spmd

f32 = mybir.dt.float32
f32r = mybir.dt.float32r

B, C_IN, H, W = 256, 2, 15, 15
DIM_MID, DIM_OUT = 128, 64
N_RES = 4
N_CORES = 8
NB = B // N_CORES            # 32 images per core
PH, PW = H + 2, W + 2        # 17x17 padded image
PAD = PH * PW                # 289
SPAN = NB * PAD              # 9248
G = 18                       # head guard (shifted AP offsets stay >= 0)
GT = 20                      # tail guard (junk col reads past last image)
BUF = G + SPAN + GT
NI = NB * H * W              # 7200 interior elems per partition
CHUNK = 2 * H * W            # 450 = 2 images per flat matmul chunk
WCHUNK = 2 * H * 16          # 480 = 2 images of 16-wide conv rhs
NCHUNK = NI // CHUNK         # 16 chunks
GRP = 4                      # chunks per PSUM-tile group (4 banks)
NGRP = NCHUNK // GRP         # 4 groups of 8 images

# tap positions in the 3x3 kernel per direction (matches reference _POS):
# horiz, vert, diag, anti-diag.  Tap k reads input offset (r-1, c-1).
POS = (((1, 0), (1, 1), (1, 2)),
       ((0, 1), (1, 1), (2, 1)),
       ((0, 0), (1, 1), (2, 2)),
       ((2, 0), (1, 1), (0, 2)))
OFFS = [tuple((r - 1) * PW + (c - 1) for r, c in taps) for taps in POS]

SILU = mybir.ActivationFunctionType.Silu


def _imgs(t, off=0):
    """[128, NB, 17, 17] padded-image view of stream buffer, shifted."""
    s = G + off
    return t[:, s:s + SPAN].rearrange("p (i h w) -> p i h w", i=NB, h=PH, w=PW)


def _conv_rhs(t, off, c):
    """[128, 2, 15, 16] moving operand for chunk c (images 2c, 2c+1):
    15 interior rows, 16 cols (last col junk, even count for fp32r)."""
    return _imgs(t, off)[:, 2 * c:2 * c + 2, 1:16, 1:17]


def _valid(ps, parts=DIM_MID):
    """[p, 4, 30, 15] valid-column view of 4 PSUM banks of 480."""
    return ps[0:parts, :, 0:WCHUNK].rearrange(
        "p b (rr c) -> p b rr c", rr=2 * H, c=16)[:, :, :, 0:15]


def _int8(t, g):
    """[128, 8, 15, 15] interior of image group g."""
    return _imgs(t)[:, 8 * g:8 * g + 8, 1:16, 1:16]


def build_program(act_func=SILU, repeats=1):
    nc = bacc.Bacc("TRN2", target_bir_lowering=False, debug=False)

    x3_d = nc.dram_tensor("x3", [4, 6, NI], f32r, kind="ExternalInput").ap()
    w03_d = nc.dram_tensor("w03", [6, DIM_MID], f32r, kind="ExternalInput").ap()
    b0_d = nc.dram_tensor("b0", [DIM_MID, 1], f32, kind="ExternalInput").ap()
    rbw_d = nc.dram_tensor("rbw", [N_RES, 3, DIM_MID, DIM_MID], f32r, kind="ExternalInput").ap()
    rbb_d = nc.dram_tensor("rbb", [N_RES, DIM_MID, 1], f32, kind="ExternalInput").ap()
    rbc1_d = nc.dram_tensor("rbc1", [N_RES, DIM_MID, DIM_MID], f32r, kind="ExternalInput").ap()
    rbc1b_d = nc.dram_tensor("rbc1b", [N_RES, DIM_MID, 1], f32, kind="ExternalInput").ap()
    c0w1_d = nc.dram_tensor("c0w1", [DIM_MID, DIM_MID], f32r, kind="ExternalInput").ap()
    c0b1_d = nc.dram_tensor("c0b1", [DIM_MID, 1], f32, kind="ExternalInput").ap()
    c0w2_d = nc.dram_tensor("c0w2", [DIM_MID, DIM_MID], f32r, kind="ExternalInput").ap()
    c0b2_d = nc.dram_tensor("c0b2", [DIM_MID, 1], f32, kind="ExternalInput").ap()
    finw_d = nc.dram_tensor("finw", [DIM_MID, DIM_OUT], f32r, kind="ExternalInput").ap()
    finb_d = nc.dram_tensor("finb", [DIM_OUT, 1], f32, kind="ExternalInput").ap()
    zb_d = nc.dram_tensor("zbuf", [DIM_MID, BUF], f32r, kind="ExternalInput").ap()
    out_d = nc.dram_tensor("out", [4, DIM_OUT, NI], f32, kind="ExternalOutput").ap()

    with tile.TileContext(nc) as tc:
        with (
            tc.tile_pool(name="const", bufs=1) as cpool,
            tc.tile_pool(name="stream", bufs=1) as spool,
            tc.tile_pool(name="x3p", bufs=1) as x3pool,
            tc.tile_pool(name="psum", bufs=2, space="PSUM") as psum,
        ):
            # ---- constants (loaded once) ----
            w03 = cpool.tile([6, DIM_MID], f32r)
            rbw = cpool.tile([DIM_MID, N_RES * 3, DIM_MID], f32r)
            rbc1 = cpool.tile([DIM_MID, N_RES, DIM_MID], f32r)
            c0w1 = cpool.tile([DIM_MID, DIM_MID], f32r)
            c0w2 = cpool.tile([DIM_MID, DIM_MID], f32r)
            finw = cpool.tile([DIM_MID, DIM_OUT], f32r)
            b0 = cpool.tile([DIM_MID, 1], f32)
            rbb = cpool.tile([DIM_MID, N_RES], f32)
            rbc1b = cpool.tile([DIM_MID, N_RES], f32)
            c0b1 = cpool.tile([DIM_MID, 1], f32)
            c0b2 = cpool.tile([DIM_MID, 1], f32)
            finb = cpool.tile([DIM_OUT, 1], f32)

            nc.sync.dma_start(w03[:], w03_d)
            nc.sync.dma_start(rbw[:], rbw_d.rearrange("i k ci co -> ci (i k) co"))
            nc.sync.dma_start(rbc1[:], rbc1_d.rearrange("i ci co -> ci i co"))
            nc.sync.dma_start(c0w1[:], c0w1_d)
            nc.sync.dma_start(c0w2[:], c0w2_d)
            nc.sync.dma_start(finw[:], finw_d)
            nc.sync.dma_start(b0[:], b0_d)
            nc.sync.dma_start(rbb[:], rbb_d.rearrange("i p one -> p (i one)"))
            nc.sync.dma_start(rbc1b[:], rbc1b_d.rearrange("i p one -> p (i one)"))
            nc.sync.dma_start(c0b1[:], c0b1_d)
            nc.sync.dma_start(c0b2[:], c0b2_d)
            nc.sync.dma_start(finb[:], finb_d)

            # ---- streams ----
            s_t = spool.tile([DIM_MID, BUF], f32r)  # residual stream (padded)
            u_t = spool.tile([DIM_MID, NI], f32r)   # block branch (flat)
            t_t = spool.tile([DIM_MID, NI], f32r)   # dconv-out (flat)
            o_t = spool.tile([DIM_OUT, NI], f32)    # final out (flat)

            # zero the stream buffer (ring + guards read by shifted taps)
            nc.sync.dma_start(s_t[:], zb_d)

            def act_group(dst, src, bias_ap):
                nc.scalar.activation(dst, src, act_func, bias=bias_ap)

            for d in range(4 * repeats):
                d = d % 4
                offs = OFFS[d]
                x3_t = x3pool.tile([6, NI], f32r)
                nc.sync.dma_start(x3_t[:], x3_d[d])

                # dconv0: K=6 matmul over pre-shifted input copies
                for g in range(NGRP):
                    ps = psum.tile([DIM_MID, GRP, 512], f32, tag="ps")
                    for j in range(GRP):
                        c = GRP * g + j
                        nc.tensor.matmul(
                            ps[:, j, 0:CHUNK], w03[:],
                            x3_t[:, c * CHUNK:(c + 1) * CHUNK])
                    act_group(_int8(s_t, g), ps[:, :, 0:CHUNK], b0[:])

                # 4 directional-conv res blocks
                for i in range(N_RES):
                    for g in range(NGRP):
                        ps = psum.tile([DIM_MID, GRP, 512], f32, tag="ps")
                        for j in range(GRP):
                            c = GRP * g + j
                            for k in range(3):
                                nc.tensor.matmul(
                                    ps[:, j, 0:WCHUNK], rbw[:, 3 * i + k, :],
                                    _conv_rhs(s_t, offs[k], c),
                                    start=(k == 0), stop=(k == 2))
                        act_group(t_t[:, g * GRP * CHUNK:(g + 1) * GRP * CHUNK],
                                  _valid(ps), rbb[:, i:i + 1])
                    for g in range(NGRP):
                        ps = psum.tile([DIM_MID, GRP, 512], f32, tag="ps")
                        for j in range(GRP):
                            c = GRP * g + j
                            nc.tensor.matmul(
                                ps[:, j, 0:CHUNK], rbc1[:, i, :],
                                t_t[:, c * CHUNK:(c + 1) * CHUNK])
                        act_group(u_t[:, g * GRP * CHUNK:(g + 1) * GRP * CHUNK],
                                  ps[:, :, 0:CHUNK], rbc1b[:, i:i + 1])
                        nc.vector.tensor_add(
                            _int8(s_t, g), _int8(s_t, g),
                            u_t[:, g * GRP * CHUNK:(g + 1) * GRP * CHUNK].rearrange(
                                "p (i h w) -> p i h w", i=8, h=H, w=W))

                # Conv0dResBlock: two pointwise layers + residual
                for g in range(NGRP):
                    ps = psum.tile([DIM_MID, GRP, 512], f32, tag="ps")
                    for j in range(GRP):
                        c = GRP * g + j
                        nc.tensor.matmul(
                            ps[:, j, 0:WCHUNK], c0w1[:],
                            _conv_rhs(s_t, 0, c))
                    act_group(t_t[:, g * GRP * CHUNK:(g + 1) * GRP * CHUNK],
                              _valid(ps), c0b1[:])
                for g in range(NGRP):
                    ps = psum.tile([DIM_MID, GRP, 512], f32, tag="ps")
                    for j in range(GRP):
                        c = GRP * g + j
                        nc.tensor.matmul(
                            ps[:, j, 0:CHUNK], c0w2[:],
                            t_t[:, c * CHUNK:(c + 1) * CHUNK])
                    act_group(u_t[:, g * GRP * CHUNK:(g + 1) * GRP * CHUNK],
                              ps[:, :, 0:CHUNK], c0b2[:])
                    nc.vector.tensor_add(
                        _int8(s_t, g), _int8(s_t, g),
                        u_t[:, g * GRP * CHUNK:(g + 1) * GRP * CHUNK].rearrange(
                            "p (i h w) -> p i h w", i=8, h=H, w=W))

                # final 1x1 conv (64 out channels) + bias
                for g in range(NGRP):
                    ps = psum.tile([DIM_MID, GRP, 512], f32, tag="ps")
                    for j in range(GRP):
                        c = GRP * g + j
                        nc.tensor.matmul(
                            ps[0:DIM_OUT, j, 0:WCHUNK], finw[:],
                            _conv_rhs(s_t, 0, c))
                    nc.vector.tensor_scalar_add(
                        o_t[:, g * GRP * CHUNK:(g + 1) * GRP * CHUNK],
                        _valid(ps, DIM_OUT), finb[:])
                nc.sync.dma_start(out_d[d], o_t[:])

    nc.compile()
    return nc


def prep_shared_inputs(dconv0_w, dconv0_b, rb_dconv_w, rb_dconv_b, rb_c1_w,
                       rb_c1_b, c0_w1, c0_b1, c0_w2, c0_b2, final_w, final_b):
    """Host-side weight transposes shared by all cores."""
    f = np.float32
    w03 = np.ascontiguousarray(
        np.asarray(dconv0_w, f).transpose(0, 2, 1).reshape(6, DIM_MID))
    return {
        "w03": _round_f32r(w03),
        "b0": np.asarray(dconv0_b, f).reshape(DIM_MID, 1),
        "rbw": _round_f32r(np.ascontiguousarray(
            np.asarray(rb_dconv_w, f).transpose(0, 1, 3, 2))),
        "rbb": np.asarray(rb_dconv_b, f).reshape(N_RES, DIM_MID, 1),
        "rbc1": _round_f32r(np.ascontiguousarray(
            np.asarray(rb_c1_w, f).transpose(0, 2, 1))),
        "rbc1b": np.asarray(rb_c1_b, f).reshape(N_RES, DIM_MID, 1),
        "c0w1": _round_f32r(np.ascontiguousarray(np.asarray(c0_w1, f).T)),
        "c0b1": np.asarray(c0_b1, f).reshape(DIM_MID, 1),
        "c0w2": _round_f32r(np.ascontiguousarray(np.asarray(c0_w2, f).T)),
        "c0b2": np.asarray(c0_b2, f).reshape(DIM_MID, 1),
        "finw": _round_f32r(np.ascontiguousarray(np.asarray(final_w, f).T)),
        "finb": np.asarray(final_b, f).reshape(DIM_OUT, 1),
        "zbuf": np.zeros((DIM_MID, BUF), f),
    }


def _round_f32r(a):
    """RNE-round fp32 to fp32r (1s+8e+11m, top 20 bits) on host, so HW
    truncation of the low 12 bits loses nothing."""
    v = np.ascontiguousarray(a, np.float32).view(np.uint32)
    r = (v + 0x7FF + ((v >> 12) & 1)) & np.uint32(0xFFFFF000)
    return r.astype(np.uint32).view(np.float32)


def prep_x3(x_shard):
    """[NB, 2, 15, 15] -> [4, 6, NI] pre-shifted interior copies."""
    P = np.zeros((NB, C_IN, PH, PW), np.float32)
    P[:, :, 1:16, 1:16] = x_shard
    x3 = np.empty((4, 6, NI), np.float32)
    for d, taps in enumerate(POS):
        for k, (sr, sc) in enumerate(taps):
            sh = P[:, :, sr:sr + H, sc:sc + W]          # [NB, 2, 15, 15]
            x3[d, 2 * k:2 * k + 2] = sh.transpose(1, 0, 2, 3).reshape(C_IN, NI)
    return _round_f32r(x3)


_CACHE = {}


def kernel(**inputs):
    if "nc" not in _CACHE:
        _CACHE["nc"] = build_program()
    nc = _CACHE["nc"]

    x = np.asarray(inputs["x"], np.float32)
    shared = prep_shared_inputs(**{k: v for k, v in inputs.items() if k != "x"})

    in_maps = []
    for c in range(N_CORES):
        shard = x[c * NB:(c + 1) * NB]
        m = dict(shared)
        m["x3"] = prep_x3(shard)
        in_maps.append(m)

    res = run_bass_kernel_spmd(nc, in_maps, core_ids=list(range(N_CORES)))

    out = np.empty((B, 4, DIM_OUT, H, W), np.float32)
    for c in range(N_CORES):
        oc = res.results[c]["out"].reshape(4, DIM_OUT, NB, H, W)
        out[c * NB:(c + 1) * NB] = oc.transpose(2, 0, 1, 3, 4)
    return out
